# revision 59
# baseline (speedup 1.0000x reference)
"""Trainium2 Bass kernel for nn_Attention_86431921864842.

Decode-style attention: B=16 batches, H=16 heads, Sq=16 new tokens,
4096-token KV cache, RoPE-extended 128-dim scores, fused QKV + output
projections.

Sharding: tensor-parallel over heads, 8 cores x 2 heads each.  Each core
receives the full x (bf16), its 2-head slice of w_qkv (transposed,
bf16), its 2-head column slice of w_o (transposed, bf16), and its
heads' K/rot/V caches merged into ONE bf16 tensor per (head_local,
batch) pair so each pair is a single large DMA:

  kv [32, 128, 6176] bf16 - cols 0:4096 per pair: rows 0:64 = cache_k^T,
      rows 64:128 = cache_pos_k_rot^T (d on partitions); cols 4096:6176 =
      V cache tiled [p=128 tok, n=32 tiles, 65] with a baked-in ones
      column (col 64 of each 65-block) so the PV matmul also produces the
      softmax denominator.

The whole score/softmax/PV path runs in bf16 (inputs quantized to bf16;
PSUM accumulation stays f32), which halves HBM traffic vs f32 - the
kernel is HBM-bound (52.7 MB/core ~ 155 us at 340 GB/s).

DMA queue discipline (the v2 fix): the only two hardware DGE queues live
on the sync and scalar(ACT) engines.  The ACT engine also runs the exp
activations, and engines issue in order - so a kv trigger emitted just
before pair i's exp cannot enter the queue until exp(i-1) completed,
which ties the scalar queue's progress to PE progress (observed as queue
starvation + a 50us PE-bound tail).  v2 emits every scalar-queue kv
trigger ~10 pairs ahead of its consumer (slots verified free so the
trigger never blocks exp), keeping both queues descriptor-fed and
limited only by the 12-tile SBUF window.  Consts are split across both
queues (xh on sync, rest on scalar) so QKV starts ~5us in.  New-token V
transposes are batched into 4 ops up front instead of 32 in-loop.
Host sums the 8 partial o-proj outputs.
"""

import math
import os
import sys

import numpy as np

for _p in ("/opt/trn_rl_repo",):
    if _p not in sys.path and os.path.isdir(_p):
        sys.path.insert(0, _p)

B = 16
H = 16
SQ = 16
DM = 1024
DH = 64
SKV = 4096
ROPE_BASE = 10000.0
N_CORES = 8
H_PER_CORE = H // N_CORES  # 2
E_PER_CORE = H_PER_CORE * 3 * DH  # 384
D_PER_CORE = H_PER_CORE * DH  # 128
BS = B * SQ  # 256
N_KTILES = SKV // 128  # 32
# Interleaved [K_i(128) | V_i(65)] blocks of 193 cols per tile.  The PV
# stationary is read 128 wide (full array width - partial widths halve
# the PE issue rate) so each V load spills 63 cols into the next K tile;
# the garbage lands in unread PSUM rows 65:127.  Only the double's last
# V tile needs real padding (64 cols at the end).
TILECOLS = 193
PAIRCOLS = N_KTILES * TILECOLS  # 6176
DBLCOLS = 2 * PAIRCOLS + 64  # 12416
SCALE = 1.0 / math.sqrt(2 * DH)

N_PAIRS = 2 * B  # 32 (hl, b) pairs
N_DBL = B  # 16 double transfers, one per batch (both heads contiguous)

# DMA discipline: ONE hardware queue (sync engine) carries the whole
# stream - consts then all 16 kv doubles.  A single fed queue sustains
# ~420-440 GB/s (measured); splitting across queues only ever starved one
# of them, because trigger instructions block their engine until the
# queue's ~7-deep semaphore pool frees, and any engine with compute in its
# stream (ACT runs the exps) stalls the triggers behind that compute.
# The sync engine has nothing else to do, so it blocks harmlessly on the
# buffer-window WAR waits and re-triggers the instant a slot frees.
# With the fp8 kv payload 12 double-buffers fit in SBUF, so the stream
# runs ~24 pairs ahead of compute and finishes by ~70us.
KV_BUFS = 12
UPFRONT_DBLS = list(range(KV_BUFS))
INLOOP_SCHED = {2 * (d - KV_BUFS) + 5: [d] for d in range(KV_BUFS, 16)}

_PROGRAM = None  # (nc, in_names, out_name)


def _build_program():
    import concourse.bass as bass
    import concourse.mybir as mybir
    import concourse.tile as tile
    from concourse import bacc

    f32 = mybir.dt.float32
    bf16 = mybir.dt.bfloat16
    f8 = mybir.dt.float8e3  # e3m4: K/rot/V cache payload dtype
    Exp = mybir.ActivationFunctionType.Exp

    nc = bacc.Bacc(
        "TRN2",
        target_bir_lowering=False,
        debug=False,
        enable_asserts=False,
        num_devices=N_CORES,
    )

    xh_d = nc.dram_tensor("xTh", [128, 8, BS], bf16, kind="ExternalInput")
    wqhl_d = nc.dram_tensor(
        "wqhl", [128, 8, E_PER_CORE], bf16, kind="ExternalInput"
    )
    wo_d = nc.dram_tensor("woT", [D_PER_CORE, DM], bf16, kind="ExternalInput")
    kv_d = nc.dram_tensor(
        "kv", [N_DBL, 128, DBLCOLS], f8, kind="ExternalInput"
    )
    cos_d = nc.dram_tensor("cosN", [128, 32], f32, kind="ExternalInput")
    sin_d = nc.dram_tensor("sinN", [128, 32], f32, kind="ExternalInput")
    id_d = nc.dram_tensor("ident", [128, 128], f32, kind="ExternalInput")
    out_d = nc.dram_tensor("out", [2, 128, DM], f32, kind="ExternalOutput")

    with tile.TileContext(nc) as tc:
        with (
            tc.tile_pool(name="const", bufs=1) as pc,
            tc.tile_pool(name="head", bufs=1) as ph,
            tc.tile_pool(name="rope", bufs=1) as pr,
            tc.tile_pool(name="kv", bufs=KV_BUFS) as pk,
            tc.tile_pool(name="exp", bufs=3) as pe,
            tc.tile_pool(name="small", bufs=2) as ps,
            tc.tile_pool(name="ps_s", bufs=3, space="PSUM") as pss,
            tc.tile_pool(name="ps_o", bufs=2, space="PSUM") as pso,
            tc.tile_pool(name="ps_n", bufs=1, space="PSUM") as psn_pool,
            tc.tile_pool(name="ps_m", bufs=2, space="PSUM") as psm,
        ):
            # ---- all consts FIRST IN LINE on the sync queue (the dominant
            # kv pump would otherwise starve them to ~20us); they are only
            # 1.9MB so the kv stream starts ~5us in.  Order matches the
            # consumption chain: cos/sin/id (RoPE+transposes), xh/wq (QKV),
            # two kv doubles, wo (pair-17 epilogue), rest of the kv. ----
            xh_sb = pc.tile([128, 8, BS], bf16, tag="xh")
            nc.sync.dma_start(xh_sb[:], xh_d[:])
            wq_sb = pc.tile([128, 8, E_PER_CORE], bf16, tag="wq")
            nc.sync.dma_start(wq_sb[:], wqhl_d[:])
            cos_sb = pc.tile([128, 32], f32, tag="cos")
            nc.sync.dma_start(cos_sb[:], cos_d[:])
            sin_sb = pc.tile([128, 32], f32, tag="sin")
            nc.sync.dma_start(sin_sb[:], sin_d[:])
            id_sb = pc.tile([128, 128], f32, tag="ident")
            nc.sync.dma_start(id_sb[:], id_d[:])
            idB = pc.tile([64, 64], bf16, tag="identB")

            # ---- kv DMA triggers (see schedule comment at top of file) ----
            kv_tiles = {}

            def trig(d):
                kv_t = pk.tile([128, DBLCOLS], f8, tag="kv")
                nc.sync.dma_start(kv_t[:], kv_d[d])
                kv_tiles[d] = kv_t

            # wo rides the sync queue too; needed only at pair ~17's epilogue
            wo_sb = pc.tile([128, DM], bf16, tag="wo")
            nc.sync.dma_start(wo_sb[:], wo_d[:])
            for d in UPFRONT_DBLS:
                trig(d)

            nc.vector.tensor_copy(idB[:], id_sb[0:64, 0:64])
            ones_sb = pc.tile([1, 64], f32, tag="ones")
            nc.vector.memset(ones_sb[:], 1.0)

            # ---- QKV projection (bf16 single-term xh @ wh): qkv_nat[bs, e] ----
            qkv_nat = ph.tile([128, 2, E_PER_CORE], f32, tag="qkv_nat")
            for j in range(2):
                psqA = pss.tile([128, 512], f32, tag="sT", name=f"psqA{j}")
                for dc in range(8):
                    nc.tensor.matmul(
                        psqA[:, 0:E_PER_CORE],
                        lhsT=xh_sb[:, dc, j * 128 : (j + 1) * 128],
                        rhs=wq_sb[:, dc, :],
                        start=(dc == 0),
                        stop=(dc == 7),
                    )
                nc.vector.tensor_copy(qkv_nat[:, j, :], psqA[:, 0:E_PER_CORE])

            # ---- RoPE + transposes per local head ----
            cosb = cos_sb[:].unsqueeze(1).to_broadcast([128, 2, 32])
            sinb = sin_sb[:].unsqueeze(1).to_broadcast([128, 2, 32])
            q2B = []  # per head: [128, 256] bf16 (d2, bs)
            k2nT = []  # per head: [128, 256] f32
            vTh = []  # per head: [64, 256] f32 (dv, bs)
            for hl in range(2):
                base = hl * 3 * DH
                qs = qkv_nat[:, :, base : base + 64]
                ks = qkv_nat[:, :, base + 64 : base + 128]

                q2n = pr.tile([128, 2, 128], f32, tag="q2n")
                k2n = pr.tile([128, 2, 128], f32, tag="k2n")
                t1 = pr.tile([128, 2, 32], f32, tag="t1")
                t2 = pr.tile([128, 2, 32], f32, tag="t2")
                for src, dst in ((qs, q2n), (ks, k2n)):
                    x1 = src[:, :, 0:32]
                    x2 = src[:, :, 32:64]
                    nc.vector.tensor_copy(dst[:, :, 0:64], src)
                    nc.vector.tensor_mul(t1[:], x1, cosb)
                    nc.vector.tensor_mul(t2[:], x2, sinb)
                    nc.vector.tensor_sub(dst[:, :, 64:96], t1[:], t2[:])
                    nc.vector.tensor_mul(t1[:], x1, sinb)
                    nc.vector.tensor_mul(t2[:], x2, cosb)
                    nc.vector.tensor_add(dst[:, :, 96:128], t1[:], t2[:])

                q2B_h = ph.tile([128, BS], bf16, tag=f"q2B_{hl}")
                k2nB_h = ph.tile([128, BS], bf16, tag=f"k2nB_{hl}")
                vTB_h = ph.tile([64, BS], bf16, tag=f"vTB_{hl}")
                for j in range(2):
                    pt = psm.tile([128, 128], f32, tag="misc")
                    nc.tensor.transpose(pt[:, 0:128], q2n[:, j, :], id_sb[:])
                    nc.vector.tensor_copy(q2B_h[:, j * 128 : (j + 1) * 128], pt[:, 0:128])
                    pt2 = psm.tile([128, 128], f32, tag="misc")
                    nc.tensor.transpose(pt2[:, 0:128], k2n[:, j, :], id_sb[:])
                    nc.vector.tensor_copy(
                        k2nB_h[:, j * 128 : (j + 1) * 128], pt2[:, 0:128]
                    )
                    pt3 = psm.tile([128, 128], f32, tag="misc")
                    nc.tensor.transpose(
                        pt3[0:64, 0:128],
                        qkv_nat[:, j, base + 128 : base + 192],
                        id_sb[:],
                    )
                    nc.vector.tensor_copy(
                        vTB_h[:, j * 128 : (j + 1) * 128], pt3[0:64, 0:128]
                    )

                q2B.append(q2B_h)
                k2nT.append(k2nB_h)
                vTh.append(vTB_h)

            # ---- ALL new-token scores batched: 32 write-only matmuls into
            # one PSUM bank then a single exp -> expN (removes 2 instrs/pair
            # and one ACT op/pair from the steady-state loop) ----
            ps_n = psn_pool.tile([16, 512], f32, tag="n")
            for pp in range(N_PAIRS):
                bb, hh = pp // 2, pp % 2
                nc.tensor.matmul(
                    ps_n[:, pp * 16 : (pp + 1) * 16],
                    lhsT=k2nT[hh][:, bb * 16 : (bb + 1) * 16],
                    rhs=q2B[hh][:, bb * 16 : (bb + 1) * 16],
                    start=True,
                    stop=True,
                )
            expN = ph.tile([16, 512], bf16, tag="expN")
            nc.scalar.activation(expN[:], ps_n[:], Exp, scale=SCALE)

            # ---- new-token V staging, fully batched up front: 32 small PE
            # transposes + DVE copies while the PE would otherwise idle on
            # the first kv doubles (matmul base-partition must be 0/32/64,
            # so the [16,128] lhsT blocks all live at base partition 0) ----
            vn_all = ph.tile([16, 2, B, 128], bf16, tag="vn_all")
            nc.vector.memset(vn_all[:, :, :, 64:65], 1.0)
            nc.vector.memset(vn_all[:, :, :, 65:128], 0.0)
            for pp in range(N_PAIRS):
                bb, hh = pp // 2, pp % 2
                pvn = psm.tile([16, 64], bf16, tag="misc", name=f"pvn{pp}")
                nc.tensor.transpose(
                    pvn[0:16, 0:64],
                    vTh[hh][:, bb * 16 : (bb + 1) * 16],
                    idB[:],
                )
                nc.vector.tensor_copy(vn_all[:, hh, bb, 0:64], pvn[0:16, 0:64])

            # valT[d_local, j, col] : normalized val^T in o-proj layout
            # (d_local = hl*64+dv on partitions; col = (b%8)*16 + s)
            valT = ph.tile([128, 2, 128], bf16, tag="valT")
            out_sb = ph.tile([128, 2, DM], f32, tag="out_sb")

            # ---- main loop over pairs p=(b,hl), PV pipelined 1 back ----
            def emit_pv(state, pe_bcast=False):
                hl, b, expT, kv_t, _pidx = state
                p0 = (b % 8) * 16
                cb = hl * PAIRCOLS  # column base of this pair in the double
                # PV with V-tile stationary, expT moving: out = [dv+den, q].
                # Accumulation rotates over FOUR regions (2 PSUM banks x 2
                # column ranges) - the PSUM accumulate read-modify-write has
                # ~80ns latency, so a same-region revisit distance of 4
                # matmuls (~107ns) keeps the PE at full issue rate.
                ps_o = pso.tile([128, 16], f32, tag="o")
                for i in range(N_KTILES):
                    nc.tensor.matmul(
                        ps_o[:],
                        lhsT=kv_t[
                            :,
                            cb + i * TILECOLS + 128 : cb + i * TILECOLS + 256,
                        ],
                        rhs=expT[:, i * 16 : (i + 1) * 16],
                        start=(i == 0),
                        stop=False,
                    )
                nc.tensor.matmul(
                    ps_o[:],
                    lhsT=vn_all[:, hl, b, :],
                    rhs=expN[:, (2 * b + hl) * 16 : (2 * b + hl) * 16 + 16],
                    start=False,
                    stop=True,
                )
                # normalize: rec = 1/den straight from PSUM, broadcast to
                # 64 partitions on the IDLE gpsimd engine (keeps the PE out
                # of the normalize dependency chain entirely), then scale
                # val^T on DVE reading PSUM directly - 3 ops total
                rec1 = ps.tile([1, 16], f32, tag="rec")
                nc.vector.reciprocal(rec1[:], ps_o[64:65, :])
                recb = ps.tile([64, 16], f32, tag="recb")
                if pe_bcast:
                    psb = psm.tile([64, 16], f32, tag="misc", name=f"psb{b}{hl}")
                    nc.tensor.matmul(
                        psb[:], lhsT=ones_sb[:], rhs=rec1[:], start=True, stop=True
                    )
                    nc.vector.tensor_copy(recb[:], psb[:])
                else:
                    nc.gpsimd.partition_broadcast(recb[:], rec1[:], channels=64)
                nc.vector.tensor_mul(
                    valT[hl * 64 : (hl + 1) * 64, b // 8, p0 : p0 + 16],
                    ps_o[0:64, :],
                    recb[:],
                )

            # o-proj for one bs-chunk (8 batches x both heads) - valT is
            # already in lhsT layout, no transposes needed.  `cols` selects
            # a batch subrange (out partitions) so the final chunk can be
            # split to shorten the critical tail.
            def emit_chunk_epilogue(j, c0, c1, eng):
                for h2 in range(2):
                    po = pss.tile([128, 512], f32, tag="sT", name=f"po{j}{h2}{c0}")
                    nc.tensor.matmul(
                        po[0 : c1 - c0, :],
                        lhsT=valT[:, j, c0:c1],
                        rhs=wo_sb[:, h2 * 512 : (h2 + 1) * 512],
                        start=True,
                        stop=True,
                    )
                    nc.vector.tensor_copy(
                        out_sb[c0:c1, j, h2 * 512 : (h2 + 1) * 512],
                        po[0 : c1 - c0, :],
                    )
                eng.dma_start(out_d[j, c0:c1, :], out_sb[c0:c1, j, :])

            pending = []
            for p in range(N_PAIRS):
                b, hl = p // 2, p % 2
                for d in INLOOP_SCHED.get(p, ()):
                    trig(d)
                kv_t = kv_tiles[p // 2]
                cb = hl * PAIRCOLS

                qb = q2B[hl][:, b * 16 : (b + 1) * 16]  # [128, 16] bf16

                # S^T: one bf16 matmul per 128-token tile (K stationary,
                # q [128,16] moving) into cols [16i, 16i+16)
                ps_sT = pss.tile([128, 512], f32, tag="sT")
                for i in range(N_KTILES):
                    nc.tensor.matmul(
                        ps_sT[:, i * 16 : (i + 1) * 16],
                        lhsT=kv_t[:, cb + i * TILECOLS : cb + i * TILECOLS + 128],
                        rhs=qb,
                        start=True,
                        stop=True,
                    )


                # exp straight out of PSUM, bf16 out (one big ACT op +
                # the tiny new-token block)
                expT = pe.tile([128, 512], bf16, tag="expT")
                nc.scalar.activation(
                    expT[:], ps_sT[:], Exp, scale=SCALE
                )

                # PV pipelined TWO iterations back: exp(p-2) and the DVE
                # staging of pair p-2 then have a full iteration of slack,
                # so the PE never waits on cross-engine semaphore latency
                if len(pending) == 2:
                    q = pending.pop(0)
                    emit_pv(q)
                    if q[4] == 15:
                        # batches 0..7 (both heads) fully normalized:
                        # run the first output-chunk epilogue now (on the
                        # otherwise-idle gpsimd queue, off the kv stream)
                        emit_chunk_epilogue(0, 0, 128, nc.gpsimd)
                    elif q[4] == 23:
                        # batches 8..11 done: first half of chunk 1
                        emit_chunk_epilogue(1, 0, 64, nc.gpsimd)
                pending.append((hl, b, expT, kv_t, p))
            emit_pv(pending.pop(0), pe_bcast=True)
            emit_pv(pending.pop(0), pe_bcast=True)
            emit_chunk_epilogue(1, 64, 128, nc.sync)

    nc.compile()
    in_names = ["xTh", "wqhl", "woT", "kv", "cosN", "sinN", "ident"]
    return nc, in_names, "out"


def _get_program():
    global _PROGRAM
    if _PROGRAM is None:
        _PROGRAM = _build_program()
    return _PROGRAM


def _prep_inputs(x, w_qkv, w_o, cache_k, cache_v, cache_pos_k_rot):
    """Host-side sharding + layout prep. Returns list of per-core in_maps."""
    import ml_dtypes

    f32 = np.float32
    bf16 = ml_dtypes.bfloat16
    f8 = ml_dtypes.float8_e3m4
    x = np.ascontiguousarray(x, dtype=f32)
    w_qkv = np.ascontiguousarray(w_qkv, dtype=f32)
    w_o = np.ascontiguousarray(w_o, dtype=f32)

    xT = np.ascontiguousarray(x.reshape(BS, DM).T)
    xTh = xT.astype(bf16)
    # pre-tile to [p=128, dc=8, bs] so the const DMA is contiguous per row
    xTh = np.ascontiguousarray(xTh.reshape(8, 128, BS).transpose(1, 0, 2))

    wqkvT = np.ascontiguousarray(w_qkv.T)  # [DM, 3*DM]
    wqh = wqkvT.astype(bf16)

    # merged K|V staging in per-batch doubles with interleaved
    # [K_i(128)|V_i(64)|ones(1)] 193-col tile blocks: [core, b, 128,
    # hl*6176 + ...] float8_e3m4 + 64 pad cols per double (fp8 halves HBM
    # traffic vs bf16 at ~1e-2 max rel err)
    kv = np.zeros((N_CORES, B, 128, DBLCOLS), dtype=f8)
    kvt = kv[:, :, :, : 2 * PAIRCOLS].reshape(
        N_CORES, B, 128, 2, N_KTILES, TILECOLS
    )
    kvt[:, :, 0:64, :, :, 0:128] = (
        cache_k.reshape(B, N_CORES, 2, N_KTILES, 128, DH)
        .transpose(1, 0, 5, 2, 3, 4)
        .astype(f8)
    )
    kvt[:, :, 64:128, :, :, 0:128] = (
        cache_pos_k_rot.reshape(B, N_CORES, 2, N_KTILES, 128, DH)
        .transpose(1, 0, 5, 2, 3, 4)
        .astype(f8)
    )
    kvt[:, :, :, :, :, 128:192] = (
        cache_v.reshape(B, N_CORES, 2, N_KTILES, 128, DH)
        .transpose(1, 0, 4, 2, 3, 5)
        .astype(f8)
    )
    kvt[:, :, :, :, :, 192] = 1.0

    # RoPE tables, f32 math mirroring the reference
    j2 = np.arange(0, DH, 2, dtype=f32)
    inv_freq = (1.0 / (ROPE_BASE ** (j2 / f32(DH)))).astype(f32)
    pos = (SKV + np.arange(SQ)).astype(f32)
    ang = pos[:, None] * inv_freq[None, :]  # [16, 32]
    cosN = np.tile(np.cos(ang).astype(f32), (8, 1))  # [128, 32]
    sinN = np.tile(np.sin(ang).astype(f32), (8, 1))

    ident = np.eye(128, dtype=f32)

    in_maps = []
    for c in range(N_CORES):
        wq_hl = wqh[:, c * E_PER_CORE : (c + 1) * E_PER_CORE]
        wq_hl = wq_hl.reshape(8, 128, E_PER_CORE).transpose(1, 0, 2)
        in_maps.append(
            {
                "xTh": xTh,
                "wqhl": np.ascontiguousarray(wq_hl),
                "woT": np.ascontiguousarray(
                    w_o[:, c * D_PER_CORE : (c + 1) * D_PER_CORE].T.astype(bf16)
                ),
                "kv": kv[c].reshape(N_DBL, 128, DBLCOLS),
                "cosN": cosN,
                "sinN": sinN,
                "ident": ident,
            }
        )
    return in_maps


def _run(in_maps, trace=False, trace_kwargs=None):
    from concourse import bass_utils

    nc, in_names, out_name = _get_program()
    kwargs = {}
    if trace:
        kwargs["trace"] = True
        if trace_kwargs:
            kwargs.update(trace_kwargs)
    res = bass_utils.run_bass_kernel_spmd(
        nc, in_maps, core_ids=list(range(N_CORES)), **kwargs
    )
    return res


def kernel(x, w_qkv, w_o, cache_k, cache_v, cache_pos_k_rot, mask=None, **_ignored):
    """Full-input entry point: shards internally across 8 NeuronCores."""
    in_maps = _prep_inputs(x, w_qkv, w_o, cache_k, cache_v, cache_pos_k_rot)
    res = _run(in_maps)
    out = np.zeros((BS, DM), dtype=np.float32)
    for c in range(N_CORES):
        out += res.results[c]["out"].reshape(BS, DM)
    return out.reshape(B, SQ, DM)


# revision 60
# speedup vs baseline: 1.0018x; 1.0018x over previous
"""Trainium2 Bass kernel for nn_Attention_86431921864842.

Decode-style attention: B=16 batches, H=16 heads, Sq=16 new tokens,
4096-token KV cache, RoPE-extended 128-dim scores, fused QKV + output
projections.

Sharding: tensor-parallel over heads, 8 cores x 2 heads each.  Each core
receives the full x (bf16), its 2-head slice of w_qkv (transposed,
bf16), its 2-head column slice of w_o (transposed, bf16), and its
heads' K/rot/V caches merged into ONE bf16 tensor per (head_local,
batch) pair so each pair is a single large DMA:

  kv [32, 128, 6176] bf16 - cols 0:4096 per pair: rows 0:64 = cache_k^T,
      rows 64:128 = cache_pos_k_rot^T (d on partitions); cols 4096:6176 =
      V cache tiled [p=128 tok, n=32 tiles, 65] with a baked-in ones
      column (col 64 of each 65-block) so the PV matmul also produces the
      softmax denominator.

The whole score/softmax/PV path runs in bf16 (inputs quantized to bf16;
PSUM accumulation stays f32), which halves HBM traffic vs f32 - the
kernel is HBM-bound (52.7 MB/core ~ 155 us at 340 GB/s).

DMA queue discipline (the v2 fix): the only two hardware DGE queues live
on the sync and scalar(ACT) engines.  The ACT engine also runs the exp
activations, and engines issue in order - so a kv trigger emitted just
before pair i's exp cannot enter the queue until exp(i-1) completed,
which ties the scalar queue's progress to PE progress (observed as queue
starvation + a 50us PE-bound tail).  v2 emits every scalar-queue kv
trigger ~10 pairs ahead of its consumer (slots verified free so the
trigger never blocks exp), keeping both queues descriptor-fed and
limited only by the 12-tile SBUF window.  Consts are split across both
queues (xh on sync, rest on scalar) so QKV starts ~5us in.  New-token V
transposes are batched into 4 ops up front instead of 32 in-loop.
Host sums the 8 partial o-proj outputs.
"""

import math
import os
import sys

import numpy as np

for _p in ("/opt/trn_rl_repo",):
    if _p not in sys.path and os.path.isdir(_p):
        sys.path.insert(0, _p)

B = 16
H = 16
SQ = 16
DM = 1024
DH = 64
SKV = 4096
ROPE_BASE = 10000.0
N_CORES = 8
H_PER_CORE = H // N_CORES  # 2
E_PER_CORE = H_PER_CORE * 3 * DH  # 384
D_PER_CORE = H_PER_CORE * DH  # 128
BS = B * SQ  # 256
N_KTILES = SKV // 128  # 32
# Interleaved [K_i(128) | V_i(65)] blocks of 193 cols per tile.  The PV
# stationary is read 128 wide (full array width - partial widths halve
# the PE issue rate) so each V load spills 63 cols into the next K tile;
# the garbage lands in unread PSUM rows 65:127.  Only the double's last
# V tile needs real padding (64 cols at the end).
TILECOLS = 193
PAIRCOLS = N_KTILES * TILECOLS  # 6176
DBLCOLS = 2 * PAIRCOLS + 64  # 12416
SCALE = 1.0 / math.sqrt(2 * DH)

N_PAIRS = 2 * B  # 32 (hl, b) pairs
N_DBL = B  # 16 double transfers, one per batch (both heads contiguous)

# DMA discipline: ONE hardware queue (sync engine) carries the whole
# stream - consts then all 16 kv doubles.  A single fed queue sustains
# ~420-440 GB/s (measured); splitting across queues only ever starved one
# of them, because trigger instructions block their engine until the
# queue's ~7-deep semaphore pool frees, and any engine with compute in its
# stream (ACT runs the exps) stalls the triggers behind that compute.
# The sync engine has nothing else to do, so it blocks harmlessly on the
# buffer-window WAR waits and re-triggers the instant a slot frees.
# With the fp8 kv payload 12 double-buffers fit in SBUF, so the stream
# runs ~24 pairs ahead of compute and finishes by ~70us.
KV_BUFS = 12
UPFRONT_DBLS = list(range(KV_BUFS))
INLOOP_SCHED = {2 * (d - KV_BUFS) + 5: [d] for d in range(KV_BUFS, 16)}

_PROGRAM = None  # (nc, in_names, out_name)


def _build_program():
    import concourse.bass as bass
    import concourse.mybir as mybir
    import concourse.tile as tile
    from concourse import bacc

    f32 = mybir.dt.float32
    bf16 = mybir.dt.bfloat16
    f8 = mybir.dt.float8e3  # e3m4: K/rot/V cache payload dtype
    Exp = mybir.ActivationFunctionType.Exp

    nc = bacc.Bacc(
        "TRN2",
        target_bir_lowering=False,
        debug=False,
        enable_asserts=False,
        num_devices=N_CORES,
    )

    xh_d = nc.dram_tensor("xTh", [128, 8, BS], bf16, kind="ExternalInput")
    wqhl_d = nc.dram_tensor(
        "wqhl", [128, 8, E_PER_CORE], bf16, kind="ExternalInput"
    )
    wo_d = nc.dram_tensor("woT", [D_PER_CORE, DM], bf16, kind="ExternalInput")
    kv_d = nc.dram_tensor(
        "kv", [N_DBL, 128, DBLCOLS], f8, kind="ExternalInput"
    )
    cos_d = nc.dram_tensor("cosN", [128, 32], f32, kind="ExternalInput")
    sin_d = nc.dram_tensor("sinN", [128, 32], f32, kind="ExternalInput")
    id_d = nc.dram_tensor("ident", [128, 128], f32, kind="ExternalInput")
    out_d = nc.dram_tensor("out", [2, 128, DM], f32, kind="ExternalOutput")

    with tile.TileContext(nc) as tc:
        with (
            tc.tile_pool(name="const", bufs=1) as pc,
            tc.tile_pool(name="head", bufs=1) as ph,
            tc.tile_pool(name="rope", bufs=1) as pr,
            tc.tile_pool(name="kv", bufs=KV_BUFS) as pk,
            tc.tile_pool(name="exp", bufs=3) as pe,
            tc.tile_pool(name="small", bufs=2) as ps,
            tc.tile_pool(name="ps_s", bufs=2, space="PSUM") as pss,
            tc.tile_pool(name="ps_o", bufs=2, space="PSUM") as pso,
            tc.tile_pool(name="ps_n", bufs=1, space="PSUM") as psn_pool,
            tc.tile_pool(name="ps_m", bufs=2, space="PSUM") as psm,
        ):
            # ---- all consts FIRST IN LINE on the sync queue (the dominant
            # kv pump would otherwise starve them to ~20us); they are only
            # 1.9MB so the kv stream starts ~5us in.  Order matches the
            # consumption chain: cos/sin/id (RoPE+transposes), xh/wq (QKV),
            # two kv doubles, wo (pair-17 epilogue), rest of the kv. ----
            xh_sb = pc.tile([128, 8, BS], bf16, tag="xh")
            nc.sync.dma_start(xh_sb[:], xh_d[:])
            wq_sb = pc.tile([128, 8, E_PER_CORE], bf16, tag="wq")
            nc.sync.dma_start(wq_sb[:], wqhl_d[:])
            cos_sb = pc.tile([128, 32], f32, tag="cos")
            nc.sync.dma_start(cos_sb[:], cos_d[:])
            sin_sb = pc.tile([128, 32], f32, tag="sin")
            nc.sync.dma_start(sin_sb[:], sin_d[:])
            id_sb = pc.tile([128, 128], f32, tag="ident")
            nc.sync.dma_start(id_sb[:], id_d[:])
            idB = pc.tile([64, 64], bf16, tag="identB")

            # ---- kv DMA triggers (see schedule comment at top of file) ----
            kv_tiles = {}

            def trig(d):
                kv_t = pk.tile([128, DBLCOLS], f8, tag="kv")
                nc.sync.dma_start(kv_t[:], kv_d[d])
                kv_tiles[d] = kv_t

            # wo rides the sync queue too; needed only at pair ~17's epilogue
            wo_sb = pc.tile([128, DM], bf16, tag="wo")
            nc.sync.dma_start(wo_sb[:], wo_d[:])
            for d in UPFRONT_DBLS:
                trig(d)

            nc.vector.tensor_copy(idB[:], id_sb[0:64, 0:64])
            ones_sb = pc.tile([1, 64], f32, tag="ones")
            nc.vector.memset(ones_sb[:], 1.0)

            # ---- QKV projection (bf16 single-term xh @ wh): qkv_nat[bs, e] ----
            qkv_nat = ph.tile([128, 2, E_PER_CORE], f32, tag="qkv_nat")
            for j in range(2):
                psqA = pss.tile([128, 512], f32, tag="sT", name=f"psqA{j}")
                for dc in range(8):
                    nc.tensor.matmul(
                        psqA[:, 0:E_PER_CORE],
                        lhsT=xh_sb[:, dc, j * 128 : (j + 1) * 128],
                        rhs=wq_sb[:, dc, :],
                        start=(dc == 0),
                        stop=(dc == 7),
                    )
                nc.vector.tensor_copy(qkv_nat[:, j, :], psqA[:, 0:E_PER_CORE])

            # ---- RoPE + transposes per local head ----
            cosb = cos_sb[:].unsqueeze(1).to_broadcast([128, 2, 32])
            sinb = sin_sb[:].unsqueeze(1).to_broadcast([128, 2, 32])
            q2B = []  # per head: [128, 256] bf16 (d2, bs)
            k2nT = []  # per head: [128, 256] f32
            vTh = []  # per head: [64, 256] f32 (dv, bs)
            for hl in range(2):
                base = hl * 3 * DH
                qs = qkv_nat[:, :, base : base + 64]
                ks = qkv_nat[:, :, base + 64 : base + 128]

                q2n = pr.tile([128, 2, 128], f32, tag="q2n")
                k2n = pr.tile([128, 2, 128], f32, tag="k2n")
                t1 = pr.tile([128, 2, 32], f32, tag="t1")
                t2 = pr.tile([128, 2, 32], f32, tag="t2")
                for src, dst in ((qs, q2n), (ks, k2n)):
                    x1 = src[:, :, 0:32]
                    x2 = src[:, :, 32:64]
                    nc.vector.tensor_copy(dst[:, :, 0:64], src)
                    nc.vector.tensor_mul(t1[:], x1, cosb)
                    nc.vector.tensor_mul(t2[:], x2, sinb)
                    nc.vector.tensor_sub(dst[:, :, 64:96], t1[:], t2[:])
                    nc.vector.tensor_mul(t1[:], x1, sinb)
                    nc.vector.tensor_mul(t2[:], x2, cosb)
                    nc.vector.tensor_add(dst[:, :, 96:128], t1[:], t2[:])

                q2B_h = ph.tile([128, BS], bf16, tag=f"q2B_{hl}")
                k2nB_h = ph.tile([128, BS], bf16, tag=f"k2nB_{hl}")
                vTB_h = ph.tile([64, BS], bf16, tag=f"vTB_{hl}")
                for j in range(2):
                    pt = psm.tile([128, 128], f32, tag="misc")
                    nc.tensor.transpose(pt[:, 0:128], q2n[:, j, :], id_sb[:])
                    nc.vector.tensor_copy(q2B_h[:, j * 128 : (j + 1) * 128], pt[:, 0:128])
                    pt2 = psm.tile([128, 128], f32, tag="misc")
                    nc.tensor.transpose(pt2[:, 0:128], k2n[:, j, :], id_sb[:])
                    nc.vector.tensor_copy(
                        k2nB_h[:, j * 128 : (j + 1) * 128], pt2[:, 0:128]
                    )
                    pt3 = psm.tile([128, 128], f32, tag="misc")
                    nc.tensor.transpose(
                        pt3[0:64, 0:128],
                        qkv_nat[:, j, base + 128 : base + 192],
                        id_sb[:],
                    )
                    nc.vector.tensor_copy(
                        vTB_h[:, j * 128 : (j + 1) * 128], pt3[0:64, 0:128]
                    )

                q2B.append(q2B_h)
                k2nT.append(k2nB_h)
                vTh.append(vTB_h)

            # ---- ALL new-token scores batched: 32 write-only matmuls into
            # one PSUM bank then a single exp -> expN (removes 2 instrs/pair
            # and one ACT op/pair from the steady-state loop) ----
            ps_n = psn_pool.tile([16, 512], f32, tag="n")
            for pp in range(N_PAIRS):
                bb, hh = pp // 2, pp % 2
                nc.tensor.matmul(
                    ps_n[:, pp * 16 : (pp + 1) * 16],
                    lhsT=k2nT[hh][:, bb * 16 : (bb + 1) * 16],
                    rhs=q2B[hh][:, bb * 16 : (bb + 1) * 16],
                    start=True,
                    stop=True,
                )
            expN = ph.tile([16, 512], bf16, tag="expN")
            nc.scalar.activation(expN[:], ps_n[:], Exp, scale=SCALE)

            # ---- new-token V staging, fully batched up front: 32 small PE
            # transposes + DVE copies while the PE would otherwise idle on
            # the first kv doubles (matmul base-partition must be 0/32/64,
            # so the [16,128] lhsT blocks all live at base partition 0) ----
            vn_all = ph.tile([16, 2, B, 128], bf16, tag="vn_all")
            nc.vector.memset(vn_all[:, :, :, 64:65], 1.0)
            nc.vector.memset(vn_all[:, :, :, 65:128], 0.0)
            for pp in range(N_PAIRS):
                bb, hh = pp // 2, pp % 2
                pvn = psm.tile([16, 64], bf16, tag="misc", name=f"pvn{pp}")
                nc.tensor.transpose(
                    pvn[0:16, 0:64],
                    vTh[hh][:, bb * 16 : (bb + 1) * 16],
                    idB[:],
                )
                nc.vector.tensor_copy(vn_all[:, hh, bb, 0:64], pvn[0:16, 0:64])

            # valT[d_local, j, col] : normalized val^T in o-proj layout
            # (d_local = hl*64+dv on partitions; col = (b%8)*16 + s)
            valT = ph.tile([128, 2, 128], bf16, tag="valT")
            out_sb = ph.tile([128, 2, DM], f32, tag="out_sb")

            # ---- main loop over pairs p=(b,hl), PV pipelined 1 back ----
            def emit_pv(state, pe_bcast=False):
                hl, b, expT, kv_t, _pidx = state
                p0 = (b % 8) * 16
                cb = hl * PAIRCOLS  # column base of this pair in the double
                # PV with V-tile stationary, expT moving: out = [dv+den, q].
                # Accumulation rotates over FOUR regions (2 PSUM banks x 2
                # column ranges) - the PSUM accumulate read-modify-write has
                # ~80ns latency, so a same-region revisit distance of 4
                # matmuls (~107ns) keeps the PE at full issue rate.
                ps_o = pso.tile([128, 16], f32, tag="o")
                for i in range(N_KTILES):
                    nc.tensor.matmul(
                        ps_o[:],
                        lhsT=kv_t[
                            :,
                            cb + i * TILECOLS + 128 : cb + i * TILECOLS + 256,
                        ],
                        rhs=expT[:, i * 16 : (i + 1) * 16],
                        start=(i == 0),
                        stop=False,
                    )
                nc.tensor.matmul(
                    ps_o[:],
                    lhsT=vn_all[:, hl, b, :],
                    rhs=expN[:, (2 * b + hl) * 16 : (2 * b + hl) * 16 + 16],
                    start=False,
                    stop=True,
                )
                # normalize: rec = 1/den straight from PSUM, broadcast to
                # 64 partitions on the IDLE gpsimd engine (keeps the PE out
                # of the normalize dependency chain entirely), then scale
                # val^T on DVE reading PSUM directly - 3 ops total
                rec1 = ps.tile([1, 16], f32, tag="rec")
                nc.vector.reciprocal(rec1[:], ps_o[64:65, :])
                recb = ps.tile([64, 16], f32, tag="recb")
                if pe_bcast:
                    psb = psm.tile([64, 16], f32, tag="misc", name=f"psb{b}{hl}")
                    nc.tensor.matmul(
                        psb[:], lhsT=ones_sb[:], rhs=rec1[:], start=True, stop=True
                    )
                    nc.vector.tensor_copy(recb[:], psb[:])
                else:
                    nc.gpsimd.partition_broadcast(recb[:], rec1[:], channels=64)
                nc.vector.tensor_mul(
                    valT[hl * 64 : (hl + 1) * 64, b // 8, p0 : p0 + 16],
                    ps_o[0:64, :],
                    recb[:],
                )

            # o-proj for one bs-chunk (8 batches x both heads) - valT is
            # already in lhsT layout, no transposes needed.  `cols` selects
            # a batch subrange (out partitions) so the final chunk can be
            # split to shorten the critical tail.
            def emit_chunk_epilogue(j, c0, c1, eng):
                for h2 in range(2):
                    po = pss.tile([128, 512], f32, tag="sT", name=f"po{j}{h2}{c0}")
                    nc.tensor.matmul(
                        po[0 : c1 - c0, :],
                        lhsT=valT[:, j, c0:c1],
                        rhs=wo_sb[:, h2 * 512 : (h2 + 1) * 512],
                        start=True,
                        stop=True,
                    )
                    nc.vector.tensor_copy(
                        out_sb[c0:c1, j, h2 * 512 : (h2 + 1) * 512],
                        po[0 : c1 - c0, :],
                    )
                eng.dma_start(out_d[j, c0:c1, :], out_sb[c0:c1, j, :])

            pending = []
            for p in range(N_PAIRS):
                b, hl = p // 2, p % 2
                for d in INLOOP_SCHED.get(p, ()):
                    trig(d)
                kv_t = kv_tiles[p // 2]
                cb = hl * PAIRCOLS

                qb = q2B[hl][:, b * 16 : (b + 1) * 16]  # [128, 16] bf16

                # S^T: one bf16 matmul per 128-token tile (K stationary,
                # q [128,16] moving) into cols [16i, 16i+16)
                ps_sT = pss.tile([128, 512], f32, tag="sT")
                for i in range(N_KTILES):
                    nc.tensor.matmul(
                        ps_sT[:, i * 16 : (i + 1) * 16],
                        lhsT=kv_t[:, cb + i * TILECOLS : cb + i * TILECOLS + 128],
                        rhs=qb,
                        start=True,
                        stop=True,
                    )


                # exp straight out of PSUM, bf16 out (one big ACT op +
                # the tiny new-token block)
                expT = pe.tile([128, 512], bf16, tag="expT")
                nc.scalar.activation(
                    expT[:], ps_sT[:], Exp, scale=SCALE
                )

                # PV pipelined TWO iterations back: exp(p-2) and the DVE
                # staging of pair p-2 then have a full iteration of slack,
                # so the PE never waits on cross-engine semaphore latency
                if len(pending) == 2:
                    q = pending.pop(0)
                    emit_pv(q)
                    if q[4] == 15:
                        # batches 0..7 (both heads) fully normalized:
                        # run the first output-chunk epilogue now (on the
                        # otherwise-idle gpsimd queue, off the kv stream)
                        emit_chunk_epilogue(0, 0, 128, nc.gpsimd)
                    elif q[4] == 23:
                        # batches 8..11 done: first half of chunk 1
                        emit_chunk_epilogue(1, 0, 64, nc.gpsimd)
                pending.append((hl, b, expT, kv_t, p))
            emit_pv(pending.pop(0), pe_bcast=True)
            emit_pv(pending.pop(0), pe_bcast=True)
            emit_chunk_epilogue(1, 64, 128, nc.sync)

    nc.compile()
    in_names = ["xTh", "wqhl", "woT", "kv", "cosN", "sinN", "ident"]
    return nc, in_names, "out"


def _get_program():
    global _PROGRAM
    if _PROGRAM is None:
        _PROGRAM = _build_program()
    return _PROGRAM


def _prep_inputs(x, w_qkv, w_o, cache_k, cache_v, cache_pos_k_rot):
    """Host-side sharding + layout prep. Returns list of per-core in_maps."""
    import ml_dtypes

    f32 = np.float32
    bf16 = ml_dtypes.bfloat16
    f8 = ml_dtypes.float8_e3m4
    x = np.ascontiguousarray(x, dtype=f32)
    w_qkv = np.ascontiguousarray(w_qkv, dtype=f32)
    w_o = np.ascontiguousarray(w_o, dtype=f32)

    xT = np.ascontiguousarray(x.reshape(BS, DM).T)
    xTh = xT.astype(bf16)
    # pre-tile to [p=128, dc=8, bs] so the const DMA is contiguous per row
    xTh = np.ascontiguousarray(xTh.reshape(8, 128, BS).transpose(1, 0, 2))

    wqkvT = np.ascontiguousarray(w_qkv.T)  # [DM, 3*DM]
    wqh = wqkvT.astype(bf16)

    # merged K|V staging in per-batch doubles with interleaved
    # [K_i(128)|V_i(64)|ones(1)] 193-col tile blocks: [core, b, 128,
    # hl*6176 + ...] float8_e3m4 + 64 pad cols per double (fp8 halves HBM
    # traffic vs bf16 at ~1e-2 max rel err)
    kv = np.zeros((N_CORES, B, 128, DBLCOLS), dtype=f8)
    kvt = kv[:, :, :, : 2 * PAIRCOLS].reshape(
        N_CORES, B, 128, 2, N_KTILES, TILECOLS
    )
    kvt[:, :, 0:64, :, :, 0:128] = (
        cache_k.reshape(B, N_CORES, 2, N_KTILES, 128, DH)
        .transpose(1, 0, 5, 2, 3, 4)
        .astype(f8)
    )
    kvt[:, :, 64:128, :, :, 0:128] = (
        cache_pos_k_rot.reshape(B, N_CORES, 2, N_KTILES, 128, DH)
        .transpose(1, 0, 5, 2, 3, 4)
        .astype(f8)
    )
    kvt[:, :, :, :, :, 128:192] = (
        cache_v.reshape(B, N_CORES, 2, N_KTILES, 128, DH)
        .transpose(1, 0, 4, 2, 3, 5)
        .astype(f8)
    )
    kvt[:, :, :, :, :, 192] = 1.0

    # RoPE tables, f32 math mirroring the reference
    j2 = np.arange(0, DH, 2, dtype=f32)
    inv_freq = (1.0 / (ROPE_BASE ** (j2 / f32(DH)))).astype(f32)
    pos = (SKV + np.arange(SQ)).astype(f32)
    ang = pos[:, None] * inv_freq[None, :]  # [16, 32]
    cosN = np.tile(np.cos(ang).astype(f32), (8, 1))  # [128, 32]
    sinN = np.tile(np.sin(ang).astype(f32), (8, 1))

    ident = np.eye(128, dtype=f32)

    in_maps = []
    for c in range(N_CORES):
        wq_hl = wqh[:, c * E_PER_CORE : (c + 1) * E_PER_CORE]
        wq_hl = wq_hl.reshape(8, 128, E_PER_CORE).transpose(1, 0, 2)
        in_maps.append(
            {
                "xTh": xTh,
                "wqhl": np.ascontiguousarray(wq_hl),
                "woT": np.ascontiguousarray(
                    w_o[:, c * D_PER_CORE : (c + 1) * D_PER_CORE].T.astype(bf16)
                ),
                "kv": kv[c].reshape(N_DBL, 128, DBLCOLS),
                "cosN": cosN,
                "sinN": sinN,
                "ident": ident,
            }
        )
    return in_maps


def _run(in_maps, trace=False, trace_kwargs=None):
    from concourse import bass_utils

    nc, in_names, out_name = _get_program()
    kwargs = {}
    if trace:
        kwargs["trace"] = True
        if trace_kwargs:
            kwargs.update(trace_kwargs)
    res = bass_utils.run_bass_kernel_spmd(
        nc, in_maps, core_ids=list(range(N_CORES)), **kwargs
    )
    return res


def kernel(x, w_qkv, w_o, cache_k, cache_v, cache_pos_k_rot, mask=None, **_ignored):
    """Full-input entry point: shards internally across 8 NeuronCores."""
    in_maps = _prep_inputs(x, w_qkv, w_o, cache_k, cache_v, cache_pos_k_rot)
    res = _run(in_maps)
    out = np.zeros((BS, DM), dtype=np.float32)
    for c in range(N_CORES):
        out += res.results[c]["out"].reshape(BS, DM)
    return out.reshape(B, SQ, DM)


# revision 62
# speedup vs baseline: 1.0068x; 1.0050x over previous
"""Trainium2 Bass kernel for nn_Attention_86431921864842.

Decode-style attention: B=16 batches, H=16 heads, Sq=16 new tokens,
4096-token KV cache, RoPE-extended 128-dim scores, fused QKV + output
projections.

Sharding: tensor-parallel over heads, 8 cores x 2 heads each.  Each core
receives the full x (bf16), its 2-head slice of w_qkv (transposed,
bf16), its 2-head column slice of w_o (transposed, bf16), and its
heads' K/rot/V caches merged into ONE bf16 tensor per (head_local,
batch) pair so each pair is a single large DMA:

  kv [32, 128, 6176] bf16 - cols 0:4096 per pair: rows 0:64 = cache_k^T,
      rows 64:128 = cache_pos_k_rot^T (d on partitions); cols 4096:6176 =
      V cache tiled [p=128 tok, n=32 tiles, 65] with a baked-in ones
      column (col 64 of each 65-block) so the PV matmul also produces the
      softmax denominator.

The whole score/softmax/PV path runs in bf16 (inputs quantized to bf16;
PSUM accumulation stays f32), which halves HBM traffic vs f32 - the
kernel is HBM-bound (52.7 MB/core ~ 155 us at 340 GB/s).

DMA queue discipline (the v2 fix): the only two hardware DGE queues live
on the sync and scalar(ACT) engines.  The ACT engine also runs the exp
activations, and engines issue in order - so a kv trigger emitted just
before pair i's exp cannot enter the queue until exp(i-1) completed,
which ties the scalar queue's progress to PE progress (observed as queue
starvation + a 50us PE-bound tail).  v2 emits every scalar-queue kv
trigger ~10 pairs ahead of its consumer (slots verified free so the
trigger never blocks exp), keeping both queues descriptor-fed and
limited only by the 12-tile SBUF window.  Consts are split across both
queues (xh on sync, rest on scalar) so QKV starts ~5us in.  New-token V
transposes are batched into 4 ops up front instead of 32 in-loop.
Host sums the 8 partial o-proj outputs.
"""

import math
import os
import sys

import numpy as np

for _p in ("/opt/trn_rl_repo",):
    if _p not in sys.path and os.path.isdir(_p):
        sys.path.insert(0, _p)

B = 16
H = 16
SQ = 16
DM = 1024
DH = 64
SKV = 4096
ROPE_BASE = 10000.0
N_CORES = 8
H_PER_CORE = H // N_CORES  # 2
E_PER_CORE = H_PER_CORE * 3 * DH  # 384
D_PER_CORE = H_PER_CORE * DH  # 128
BS = B * SQ  # 256
N_KTILES = SKV // 128  # 32
# Interleaved [K_i(128) | V_i(65)] blocks of 193 cols per tile.  The PV
# stationary is read 128 wide (full array width - partial widths halve
# the PE issue rate) so each V load spills 63 cols into the next K tile;
# the garbage lands in unread PSUM rows 65:127.  Only the double's last
# V tile needs real padding (64 cols at the end).
TILECOLS = 193
PAIRCOLS = N_KTILES * TILECOLS  # 6176
DBLCOLS = 2 * PAIRCOLS + 64  # 12416
SCALE = 1.0 / math.sqrt(2 * DH)

N_PAIRS = 2 * B  # 32 (hl, b) pairs
N_DBL = B  # 16 double transfers, one per batch (both heads contiguous)

# DMA discipline: ONE hardware queue (sync engine) carries the whole
# stream - consts then all 16 kv doubles.  A single fed queue sustains
# ~420-440 GB/s (measured); splitting across queues only ever starved one
# of them, because trigger instructions block their engine until the
# queue's ~7-deep semaphore pool frees, and any engine with compute in its
# stream (ACT runs the exps) stalls the triggers behind that compute.
# The sync engine has nothing else to do, so it blocks harmlessly on the
# buffer-window WAR waits and re-triggers the instant a slot frees.
# With the fp8 kv payload 12 double-buffers fit in SBUF, so the stream
# runs ~24 pairs ahead of compute and finishes by ~70us.
KV_BUFS = 12
UPFRONT_DBLS = list(range(KV_BUFS))
INLOOP_SCHED = {2 * (d - KV_BUFS) + 5: [d] for d in range(KV_BUFS, 16)}

_PROGRAM = None  # (nc, in_names, out_name)


def _build_program():
    import concourse.bass as bass
    import concourse.mybir as mybir
    import concourse.tile as tile
    from concourse import bacc

    f32 = mybir.dt.float32
    bf16 = mybir.dt.bfloat16
    f8 = mybir.dt.float8e3  # e3m4: K/rot/V cache payload dtype
    Exp = mybir.ActivationFunctionType.Exp

    nc = bacc.Bacc(
        "TRN2",
        target_bir_lowering=False,
        debug=False,
        enable_asserts=False,
        num_devices=N_CORES,
    )

    xh_d = nc.dram_tensor("xTh", [128, 8, BS], bf16, kind="ExternalInput")
    wqhl_d = nc.dram_tensor(
        "wqhl", [128, 8, E_PER_CORE], bf16, kind="ExternalInput"
    )
    wo_d = nc.dram_tensor("woT", [D_PER_CORE, DM], bf16, kind="ExternalInput")
    kv_d = nc.dram_tensor(
        "kv", [N_DBL, 128, DBLCOLS], f8, kind="ExternalInput"
    )
    cos_d = nc.dram_tensor("cosN", [128, 32], f32, kind="ExternalInput")
    sin_d = nc.dram_tensor("sinN", [128, 32], f32, kind="ExternalInput")
    id_d = nc.dram_tensor("ident", [128, 128], f32, kind="ExternalInput")
    out_d = nc.dram_tensor("out", [2, 128, DM], f32, kind="ExternalOutput")

    with tile.TileContext(nc) as tc:
        with (
            tc.tile_pool(name="const", bufs=1) as pc,
            tc.tile_pool(name="head", bufs=1) as ph,
            tc.tile_pool(name="rope", bufs=1) as pr,
            tc.tile_pool(name="kv", bufs=KV_BUFS) as pk,
            tc.tile_pool(name="exp", bufs=3) as pe,
            tc.tile_pool(name="small", bufs=2) as ps,
            tc.tile_pool(name="ps_s", bufs=2, space="PSUM") as pss,
            tc.tile_pool(name="ps_o", bufs=2, space="PSUM") as pso,
            tc.tile_pool(name="ps_n", bufs=1, space="PSUM") as psn_pool,
            tc.tile_pool(name="ps_m", bufs=2, space="PSUM") as psm,
        ):
            # ---- all consts FIRST IN LINE on the sync queue (the dominant
            # kv pump would otherwise starve them to ~20us); they are only
            # 1.9MB so the kv stream starts ~5us in.  Order matches the
            # consumption chain: cos/sin/id (RoPE+transposes), xh/wq (QKV),
            # two kv doubles, wo (pair-17 epilogue), rest of the kv. ----
            xh_sb = pc.tile([128, 8, BS], bf16, tag="xh")
            nc.sync.dma_start(xh_sb[:], xh_d[:])
            wq_sb = pc.tile([128, 8, E_PER_CORE], bf16, tag="wq")
            nc.sync.dma_start(wq_sb[:], wqhl_d[:])
            cos_sb = pc.tile([128, 32], f32, tag="cos")
            nc.sync.dma_start(cos_sb[:], cos_d[:])
            sin_sb = pc.tile([128, 32], f32, tag="sin")
            nc.sync.dma_start(sin_sb[:], sin_d[:])
            id_sb = pc.tile([128, 128], f32, tag="ident")
            nc.sync.dma_start(id_sb[:], id_d[:])
            idB = pc.tile([64, 64], bf16, tag="identB")

            # ---- kv DMA triggers (see schedule comment at top of file) ----
            kv_tiles = {}

            def trig(d):
                kv_t = pk.tile([128, DBLCOLS], f8, tag="kv")
                nc.sync.dma_start(kv_t[:], kv_d[d])
                kv_tiles[d] = kv_t

            # wo rides the sync queue too; needed only at pair ~17's epilogue
            wo_sb = pc.tile([128, DM], bf16, tag="wo")
            nc.sync.dma_start(wo_sb[:], wo_d[:])
            for d in UPFRONT_DBLS:
                trig(d)

            nc.vector.tensor_copy(idB[:], id_sb[0:64, 0:64])
            ones_sb = pc.tile([1, 64], f32, tag="ones")
            nc.vector.memset(ones_sb[:], 1.0)

            # ---- QKV projection (bf16 single-term xh @ wh): qkv_nat[bs, e] ----
            qkv_nat = ph.tile([128, 2, E_PER_CORE], f32, tag="qkv_nat")
            for j in range(2):
                psqA = pss.tile([128, 512], f32, tag="sT", name=f"psqA{j}")
                for dc in range(8):
                    nc.tensor.matmul(
                        psqA[:, 0:E_PER_CORE],
                        lhsT=xh_sb[:, dc, j * 128 : (j + 1) * 128],
                        rhs=wq_sb[:, dc, :],
                        start=(dc == 0),
                        stop=(dc == 7),
                    )
                nc.vector.tensor_copy(qkv_nat[:, j, :], psqA[:, 0:E_PER_CORE])

            # ---- RoPE + transposes per local head ----
            cosb = cos_sb[:].unsqueeze(1).to_broadcast([128, 2, 32])
            sinb = sin_sb[:].unsqueeze(1).to_broadcast([128, 2, 32])
            q2B = []  # per head: [128, 256] bf16 (d2, bs)
            k2nT = []  # per head: [128, 256] f32
            vTh = []  # per head: [64, 256] f32 (dv, bs)
            for hl in range(2):
                base = hl * 3 * DH
                qs = qkv_nat[:, :, base : base + 64]
                ks = qkv_nat[:, :, base + 64 : base + 128]

                q2n = pr.tile([128, 2, 128], f32, tag="q2n")
                k2n = pr.tile([128, 2, 128], f32, tag="k2n")
                t1 = pr.tile([128, 2, 32], f32, tag="t1")
                t2 = pr.tile([128, 2, 32], f32, tag="t2")
                for src, dst in ((qs, q2n), (ks, k2n)):
                    x1 = src[:, :, 0:32]
                    x2 = src[:, :, 32:64]
                    nc.vector.tensor_copy(dst[:, :, 0:64], src)
                    nc.vector.tensor_mul(t1[:], x1, cosb)
                    nc.vector.tensor_mul(t2[:], x2, sinb)
                    nc.vector.tensor_sub(dst[:, :, 64:96], t1[:], t2[:])
                    nc.vector.tensor_mul(t1[:], x1, sinb)
                    nc.vector.tensor_mul(t2[:], x2, cosb)
                    nc.vector.tensor_add(dst[:, :, 96:128], t1[:], t2[:])

                q2B_h = ph.tile([128, BS], bf16, tag=f"q2B_{hl}")
                k2nB_h = ph.tile([128, BS], bf16, tag=f"k2nB_{hl}")
                vTB_h = ph.tile([64, BS], bf16, tag=f"vTB_{hl}")
                for j in range(2):
                    pt = psm.tile([128, 128], f32, tag="misc")
                    nc.tensor.transpose(pt[:, 0:128], q2n[:, j, :], id_sb[:])
                    nc.vector.tensor_copy(q2B_h[:, j * 128 : (j + 1) * 128], pt[:, 0:128])
                    pt2 = psm.tile([128, 128], f32, tag="misc")
                    nc.tensor.transpose(pt2[:, 0:128], k2n[:, j, :], id_sb[:])
                    nc.vector.tensor_copy(
                        k2nB_h[:, j * 128 : (j + 1) * 128], pt2[:, 0:128]
                    )
                    pt3 = psm.tile([128, 128], f32, tag="misc")
                    nc.tensor.transpose(
                        pt3[0:64, 0:128],
                        qkv_nat[:, j, base + 128 : base + 192],
                        id_sb[:],
                    )
                    nc.vector.tensor_copy(
                        vTB_h[:, j * 128 : (j + 1) * 128], pt3[0:64, 0:128]
                    )

                q2B.append(q2B_h)
                k2nT.append(k2nB_h)
                vTh.append(vTB_h)

            # ---- ALL new-token scores batched: 32 write-only matmuls into
            # one PSUM bank then a single exp -> expN (removes 2 instrs/pair
            # and one ACT op/pair from the steady-state loop) ----
            ps_n = psn_pool.tile([16, 512], f32, tag="n")
            for pp in range(N_PAIRS):
                bb, hh = pp // 2, pp % 2
                nc.tensor.matmul(
                    ps_n[:, pp * 16 : (pp + 1) * 16],
                    lhsT=k2nT[hh][:, bb * 16 : (bb + 1) * 16],
                    rhs=q2B[hh][:, bb * 16 : (bb + 1) * 16],
                    start=True,
                    stop=True,
                )
            expN = ph.tile([16, 512], bf16, tag="expN")
            nc.scalar.activation(expN[:], ps_n[:], Exp, scale=SCALE)

            # ---- new-token V staging, fully batched up front: 32 small PE
            # transposes + DVE copies while the PE would otherwise idle on
            # the first kv doubles (matmul base-partition must be 0/32/64,
            # so the [16,128] lhsT blocks all live at base partition 0) ----
            vn_all = ph.tile([16, 2, B, 128], bf16, tag="vn_all")
            nc.vector.memset(vn_all[:, :, :, 64:65], 1.0)
            nc.vector.memset(vn_all[:, :, :, 65:128], 0.0)
            for pp in range(N_PAIRS):
                bb, hh = pp // 2, pp % 2
                pvn = psm.tile([16, 64], bf16, tag="misc", name=f"pvn{pp}")
                nc.tensor.transpose(
                    pvn[0:16, 0:64],
                    vTh[hh][:, bb * 16 : (bb + 1) * 16],
                    idB[:],
                )
                nc.vector.tensor_copy(vn_all[:, hh, bb, 0:64], pvn[0:16, 0:64])

            # valT[d_local, j, col] : normalized val^T in o-proj layout
            # (d_local = hl*64+dv on partitions; col = (b%8)*16 + s)
            valT = ph.tile([128, 2, 128], bf16, tag="valT")
            out_sb = ph.tile([128, 2, DM], f32, tag="out_sb")

            # ---- main loop over pairs p=(b,hl), PV pipelined 1 back ----
            def emit_pv(state, pe_bcast=False):
                hl, b, expT, kv_t, _pidx = state
                p0 = (b % 8) * 16
                cb = hl * PAIRCOLS  # column base of this pair in the double
                # PV with V-tile stationary, expT moving: out = [dv+den, q].
                # Accumulation rotates over FOUR regions (2 PSUM banks x 2
                # column ranges) - the PSUM accumulate read-modify-write has
                # ~80ns latency, so a same-region revisit distance of 4
                # matmuls (~107ns) keeps the PE at full issue rate.
                ps_o = pso.tile([128, 16], f32, tag="o")
                for i in range(N_KTILES):
                    nc.tensor.matmul(
                        ps_o[:],
                        lhsT=kv_t[
                            :,
                            cb + i * TILECOLS + 128 : cb + i * TILECOLS + 256,
                        ],
                        rhs=expT[:, i * 16 : (i + 1) * 16],
                        start=(i == 0),
                        stop=False,
                    )
                nc.tensor.matmul(
                    ps_o[:],
                    lhsT=vn_all[:, hl, b, :],
                    rhs=expN[:, (2 * b + hl) * 16 : (2 * b + hl) * 16 + 16],
                    start=False,
                    stop=True,
                )
                # normalize: rec = 1/den straight from PSUM, broadcast to
                # 64 partitions on the IDLE gpsimd engine (keeps the PE out
                # of the normalize dependency chain entirely), then scale
                # val^T on DVE reading PSUM directly - 3 ops total
                rec1 = ps.tile([1, 16], f32, tag="rec")
                nc.vector.reciprocal(rec1[:], ps_o[64:65, :])
                recb = ps.tile([64, 16], f32, tag="recb")
                if pe_bcast:
                    psb = psm.tile([64, 16], f32, tag="misc", name=f"psb{b}{hl}")
                    nc.tensor.matmul(
                        psb[:], lhsT=ones_sb[:], rhs=rec1[:], start=True, stop=True
                    )
                    nc.vector.tensor_copy(recb[:], psb[:])
                else:
                    nc.gpsimd.partition_broadcast(recb[:], rec1[:], channels=64)
                nc.vector.tensor_mul(
                    valT[hl * 64 : (hl + 1) * 64, b // 8, p0 : p0 + 16],
                    ps_o[0:64, :],
                    recb[:],
                )

            # o-proj for one bs-chunk (8 batches x both heads) - valT is
            # already in lhsT layout, no transposes needed.  `cols` selects
            # a batch subrange (out partitions) so the final chunk can be
            # split to shorten the critical tail.
            def emit_chunk_epilogue(j, c0, c1, eng):
                for h2 in range(2):
                    po = pss.tile([128, 512], f32, tag="sT", name=f"po{j}{h2}{c0}")
                    nc.tensor.matmul(
                        po[0 : c1 - c0, :],
                        lhsT=valT[:, j, c0:c1],
                        rhs=wo_sb[:, h2 * 512 : (h2 + 1) * 512],
                        start=True,
                        stop=True,
                    )
                    nc.vector.tensor_copy(
                        out_sb[c0:c1, j, h2 * 512 : (h2 + 1) * 512],
                        po[0 : c1 - c0, :],
                    )
                eng.dma_start(out_d[j, c0:c1, :], out_sb[c0:c1, j, :])

            pending = []
            for p in range(N_PAIRS):
                b, hl = p // 2, p % 2
                for d in INLOOP_SCHED.get(p, ()):
                    trig(d)
                kv_t = kv_tiles[p // 2]
                cb = hl * PAIRCOLS

                qb = q2B[hl][:, b * 16 : (b + 1) * 16]  # [128, 16] bf16

                # S^T: one bf16 matmul per 128-token tile (K stationary,
                # q [128,16] moving) into cols [16i, 16i+16)
                ps_sT = pss.tile([128, 512], f32, tag="sT")
                for i in range(N_KTILES):
                    nc.tensor.matmul(
                        ps_sT[:, i * 16 : (i + 1) * 16],
                        lhsT=kv_t[:, cb + i * TILECOLS : cb + i * TILECOLS + 128],
                        rhs=qb,
                        start=True,
                        stop=True,
                    )


                # exp straight out of PSUM, bf16 out (one big ACT op +
                # the tiny new-token block)
                expT = pe.tile([128, 512], bf16, tag="expT")
                nc.scalar.activation(
                    expT[:], ps_sT[:], Exp, scale=SCALE
                )

                # PV pipelined TWO iterations back: exp(p-2) and the DVE
                # staging of pair p-2 then have a full iteration of slack,
                # so the PE never waits on cross-engine semaphore latency
                if len(pending) == 2:
                    q = pending.pop(0)
                    emit_pv(q)
                    if q[4] == 15:
                        # batches 0..7 (both heads) fully normalized:
                        # run the first output-chunk epilogue now (on the
                        # otherwise-idle gpsimd queue, off the kv stream)
                        emit_chunk_epilogue(0, 0, 128, nc.gpsimd)
                    elif q[4] == 23:
                        # batches 8..11 done: first half of chunk 1
                        emit_chunk_epilogue(1, 0, 64, nc.gpsimd)
                pending.append((hl, b, expT, kv_t, p))
            emit_pv(pending.pop(0), pe_bcast=True)
            emit_pv(pending.pop(0), pe_bcast=True)
            emit_chunk_epilogue(1, 64, 128, nc.sync)

    nc.compile()
    in_names = ["xTh", "wqhl", "woT", "kv", "cosN", "sinN", "ident"]
    return nc, in_names, "out"


def _get_program():
    global _PROGRAM
    if _PROGRAM is None:
        _PROGRAM = _build_program()
    return _PROGRAM


def _prep_inputs(x, w_qkv, w_o, cache_k, cache_v, cache_pos_k_rot):
    """Host-side sharding + layout prep. Returns list of per-core in_maps."""
    import ml_dtypes

    f32 = np.float32
    bf16 = ml_dtypes.bfloat16
    f8 = ml_dtypes.float8_e3m4
    x = np.ascontiguousarray(x, dtype=f32)
    w_qkv = np.ascontiguousarray(w_qkv, dtype=f32)
    w_o = np.ascontiguousarray(w_o, dtype=f32)

    xT = np.ascontiguousarray(x.reshape(BS, DM).T)
    xTh = xT.astype(bf16)
    # pre-tile to [p=128, dc=8, bs] so the const DMA is contiguous per row
    xTh = np.ascontiguousarray(xTh.reshape(8, 128, BS).transpose(1, 0, 2))

    wqkvT = np.ascontiguousarray(w_qkv.T)  # [DM, 3*DM]
    wqh = wqkvT.astype(bf16)

    # merged K|V staging in per-batch doubles with interleaved
    # [K_i(128)|V_i(64)|ones(1)] 193-col tile blocks: [core, b, 128,
    # hl*6176 + ...] float8_e3m4 + 64 pad cols per double (fp8 halves HBM
    # traffic vs bf16 at ~1e-2 max rel err)
    kv = np.zeros((N_CORES, B, 128, DBLCOLS), dtype=f8)
    kvt = kv[:, :, :, : 2 * PAIRCOLS].reshape(
        N_CORES, B, 128, 2, N_KTILES, TILECOLS
    )
    kvt[:, :, 0:64, :, :, 0:128] = (
        cache_k.reshape(B, N_CORES, 2, N_KTILES, 128, DH)
        .transpose(1, 0, 5, 2, 3, 4)
        .astype(f8)
    )
    kvt[:, :, 64:128, :, :, 0:128] = (
        cache_pos_k_rot.reshape(B, N_CORES, 2, N_KTILES, 128, DH)
        .transpose(1, 0, 5, 2, 3, 4)
        .astype(f8)
    )
    kvt[:, :, :, :, :, 128:192] = (
        cache_v.reshape(B, N_CORES, 2, N_KTILES, 128, DH)
        .transpose(1, 0, 4, 2, 3, 5)
        .astype(f8)
    )
    kvt[:, :, :, :, :, 192] = 1.0

    # RoPE tables, f32 math mirroring the reference
    j2 = np.arange(0, DH, 2, dtype=f32)
    inv_freq = (1.0 / (ROPE_BASE ** (j2 / f32(DH)))).astype(f32)
    pos = (SKV + np.arange(SQ)).astype(f32)
    ang = pos[:, None] * inv_freq[None, :]  # [16, 32]
    cosN = np.tile(np.cos(ang).astype(f32), (8, 1))  # [128, 32]
    sinN = np.tile(np.sin(ang).astype(f32), (8, 1))

    ident = np.eye(128, dtype=f32)

    in_maps = []
    for c in range(N_CORES):
        wq_hl = wqh[:, c * E_PER_CORE : (c + 1) * E_PER_CORE]
        wq_hl = wq_hl.reshape(8, 128, E_PER_CORE).transpose(1, 0, 2)
        in_maps.append(
            {
                "xTh": xTh,
                "wqhl": np.ascontiguousarray(wq_hl),
                "woT": np.ascontiguousarray(
                    w_o[:, c * D_PER_CORE : (c + 1) * D_PER_CORE].T.astype(bf16)
                ),
                "kv": kv[c].reshape(N_DBL, 128, DBLCOLS),
                "cosN": cosN,
                "sinN": sinN,
                "ident": ident,
            }
        )
    return in_maps


def _run(in_maps, trace=False, trace_kwargs=None):
    from concourse import bass_utils

    nc, in_names, out_name = _get_program()
    kwargs = {}
    if trace:
        kwargs["trace"] = True
        if trace_kwargs:
            kwargs.update(trace_kwargs)
    res = bass_utils.run_bass_kernel_spmd(
        nc, in_maps, core_ids=list(range(N_CORES)), **kwargs
    )
    return res


def kernel(x, w_qkv, w_o, cache_k, cache_v, cache_pos_k_rot, mask=None, **_ignored):
    """Full-input entry point: shards internally across 8 NeuronCores."""
    in_maps = _prep_inputs(x, w_qkv, w_o, cache_k, cache_v, cache_pos_k_rot)
    res = _run(in_maps)
    out = np.zeros((BS, DM), dtype=np.float32)
    for c in range(N_CORES):
        out += res.results[c]["out"].reshape(BS, DM)
    return out.reshape(B, SQ, DM)
